# revision 31
# baseline (speedup 1.0000x reference)
"""GCN (2x GCNConv + MLP head) on 8 TRN2 NeuronCores via Bass/Tile.

v4: sharded message build + split AllGathers + balanced scan aggregation.
  - Nodes owned by core in 12720-blocks. Each dst gets a SLOT via per-core
    degree-round-robin binning into 32 (stream, quarter) bins (balance_bins)
    so per-call gather counts are near the statistical floor. Table column
    of node v = 12800*c + posmap[v] EVERYWHERE (xt, m1T/m2T, h1db, agg,
    output). Slot 399 of every quarter is reserved empty -> every window
    has a zero column for gather pads (ZPAD).
  - Self-loops are never gathered: in slot layout their contribution is a
    contiguous slice of the core's LOCAL half-shards (m1shA/B, m2shA/B),
    added into agg with one DVE add per quarter at conv start.
  - Phase A computes W1^T xt for the own 12800-column block in two 6400
    halves; each half AllGathers immediately so the collective overlaps
    the other half's compute and the conv's first window. Same for phase C
    (W2^T h1d). PSUM->SBUF casts run on the Scalar engine.
  - conv: 16 windows x 4 dst-quarter calls, window tables double-buffered.
    Per call: one ap_gather (main), 4 per-quad tensor_tensor_scans
    (running masked segment sums), one extraction ap_gather (per-dst last
    positions; absent dsts point at a zeroed pad slot), one accumulate add.
    Calls software-pipelined: gather(i+1) issues before extract(i).
    ap_gather costs ~28ns/idx of serial GpSimd time (order/dtype/d
    independent) -- the dominant cost; all layout choices minimize total
    index count (edges + NWIN*SDPAD extraction slots).
  - conv1 epilogue -> h1db (dinv-prescaled bf16), conv2 epilogue -> h2b
    f32, MLP head on own shard, host reassembles via posmap.
"""

import numpy as np
import ml_dtypes

import concourse.bass as bass  # noqa: F401
import concourse.bacc as bacc
import concourse.tile as tile
import concourse.mybir as mybir
from concourse.bass_utils import run_bass_kernel_spmd

F32 = mybir.dt.float32
BF16 = mybir.dt.bfloat16
I16 = mybir.dt.int16

NCORES = 8
NSTREAM = 8
SHARD = 12720        # nodes owned per core (last core partial)
SDST = 1590          # nodes per dst-stream
SDPAD = 1600         # padded dsts per stream
NH = 4               # dst-quarter calls per window
HALF = SDPAD // NH   # 400
DPAD = SDPAD * NSTREAM          # 12800 table columns per core block
NPAD = DPAD * NCORES            # 102400 total table columns
WIN = 6400           # window width (2 windows per core block)
NWIN = NPAD // WIN   # 16
ZPAD = 399           # slot 399 of the window-first quarter is always zero
EC = 512             # column chunk for PE phases
N_FULL = 100000


def wrapg(a):
    """[8, n] per-group idx streams -> [128, n//16] int16 ap_gather layout."""
    out = np.empty((128, a.shape[1] // 16), np.int16)
    for g in range(NSTREAM):
        out[16 * g:16 * g + 16] = np.ascontiguousarray(a[g].reshape(-1, 16).T)
    return out


def col_of(v):
    """Table column of node id v (vectorized)."""
    c = v // SHARD
    l = v - c * SHARD
    g = l // SDST
    p = l - g * SDST
    return (c * DPAD + g * SDPAD + p), c, g, p


def balance_bins(n, indeg):
    """Per core, deal dsts round-robin (by in-degree, desc) into the 32
    (stream, quarter) bins; slot 399 of every quarter stays empty so each
    window has a guaranteed zero column for gather pads."""
    posmap = np.empty(n, np.int64)
    nbins = NSTREAM * NH
    for c in range(NCORES):
        lo, hi = c * SHARD, min((c + 1) * SHARD, n)
        m = hi - lo
        order_d = np.argsort(-indeg[lo:hi], kind="stable")
        b = np.arange(m) % nbins
        slot = np.arange(m) // nbins         # <= 397 < 399
        pos = (b // NH) * SDPAD + (b % NH) * HALF + slot
        pm = np.empty(m, np.int64)
        pm[order_d] = pos
        posmap[lo:hi] = pm
    return posmap


def preprocess(n, edge_index):
    """Slot-based layout: table column of node v = (v//SHARD)*DPAD +
    posmap[v] everywhere (xt, m1T, m2T, h1db, agg, output). Self-loops are
    NOT gathered -- their contribution is a contiguous slice-add from the
    core's local m1sh/m2sh. One shared metadata set serves both convs."""
    src = edge_index[0].astype(np.int64)
    dst = edge_index[1].astype(np.int64)

    deg = np.bincount(dst, minlength=n).astype(np.float64) + 1.0
    dinv = (1.0 / np.sqrt(deg)).astype(np.float32)

    indeg = np.bincount(dst, minlength=n)
    posmap = balance_bins(n, indeg)
    cpos = (np.arange(n) // SHARD) * DPAD + posmap

    srcA, dstA = src, dst                    # no self-loops in the gather
    colS = cpos[srcA]
    w_of = colS // WIN                       # 0..NWIN-1

    c_of = dstA // SHARD
    pD = posmap[dstA]                        # slot in [0, DPAD)
    g_of = pD // SDPAD
    sl = pD % SDPAD                          # position within stream
    h_of = sl // HALF                        # 0..3

    key = (((c_of * NSTREAM + g_of) * NWIN + w_of) * NH + h_of)
    order = np.lexsort((sl, key))
    srcC, slS = colS[order], sl[order]

    ncalls = NWIN * NH
    counts = np.bincount(key, minlength=NCORES * NSTREAM * ncalls)
    starts = np.zeros(len(counts) + 1, np.int64)
    np.cumsum(counts, out=starts[1:])
    cmax = counts.reshape(NCORES, NSTREAM, NWIN, NH).max(axis=(0, 1))
    csz = ((cmax + 1 + 15) // 16) * 16       # >= cmax+1: room for a pad slot
    gtot = int(csz.sum())
    etot = NWIN * SDPAD

    cores = []
    for c in range(NCORES):
        gidx = np.full((NSTREAM, gtot), ZPAD, np.int16)   # pads -> zero col
        mask = np.ones((NSTREAM, gtot), np.float32)
        eidx = np.zeros((NSTREAM, etot), np.int16)
        off = 0
        eoff = 0
        for w in range(NWIN):
            for h in range(NH):
                cs = int(csz[w, h])
                d0 = h * HALF
                for g in range(NSTREAM):
                    k = ((c * NSTREAM + g) * NWIN + w) * NH + h
                    a, b = starts[k], starts[k + 1]
                    cnt = b - a
                    d_loc = slS[a:b]
                    gidx[g, off:off + cnt] = (srcC[a:b] - w * WIN).astype(np.int16)
                    m = np.ones(cnt + 1, np.float32)
                    m[0] = 0.0
                    if cnt:
                        m[1:cnt][d_loc[1:] != d_loc[:-1]] = 0.0
                        m[cnt] = 0.0          # first pad slot resets to 0
                        lastpos = np.full(HALF, cnt, np.int64)  # absent -> pad
                        lastpos[d_loc - d0] = np.arange(cnt)    # last wins
                        eidx[g, eoff + d0:eoff + d0 + HALF] = lastpos.astype(np.int16)
                    else:
                        eidx[g, eoff + d0:eoff + d0 + HALF] = 0
                    mask[g, off:off + cnt + 1] = m
                off += cs
            eoff += SDPAD
        dinvd = np.zeros(DPAD, np.float32)
        lo, hi = c * SHARD, min((c + 1) * SHARD, n)
        dinvd[posmap[lo:hi]] = dinv[lo:hi]
        dinvd = dinvd.reshape(NSTREAM, SDPAD)
        cores.append(dict(
            gidx=wrapg(gidx),
            mask=np.repeat(mask, 16, axis=0).astype(ml_dtypes.bfloat16),
            eidx=wrapg(eidx),
            dinvd=np.repeat(dinvd, 16, axis=0),
        ))

    plan = dict(csz=csz, gtot=gtot, etot=etot, posmap=posmap)
    return dinv, plan, cores


def build_program(plan):
    csz = plan["csz"]
    gtot = plan["gtot"]
    etot = plan["etot"]
    CSMAX = int(csz.max())

    nc = bacc.Bacc("TRN2", target_bir_lowering=False, debug=False,
                   num_devices=NCORES)

    xt = nc.dram_tensor("xt", [128, DPAD], BF16, kind="ExternalInput")
    gidx_d = nc.dram_tensor("gidx", [128, gtot // 16], I16, kind="ExternalInput")
    mask_d = nc.dram_tensor("mask", [128, gtot], BF16, kind="ExternalInput")
    eidx_d = nc.dram_tensor("eidx", [128, etot // 16], I16, kind="ExternalInput")
    dinvd_d = nc.dram_tensor("dinvd", [128, SDPAD], F32, kind="ExternalInput")
    w1_d = nc.dram_tensor("w1", [128, 64], BF16, kind="ExternalInput")
    w2_d = nc.dram_tensor("w2", [64, 64], BF16, kind="ExternalInput")
    lw1_d = nc.dram_tensor("lw1", [64, 64], F32, kind="ExternalInput")
    lw2_d = nc.dram_tensor("lw2", [64, 32], F32, kind="ExternalInput")
    lw3_d = nc.dram_tensor("lw3", [32, 1], F32, kind="ExternalInput")
    b1q_d = nc.dram_tensor("b1q", [128, 4], F32, kind="ExternalInput")
    b2q_d = nc.dram_tensor("b2q", [128, 4], F32, kind="ExternalInput")
    lb1_d = nc.dram_tensor("lb1", [64, 1], F32, kind="ExternalInput")
    lb2_d = nc.dram_tensor("lb2", [32, 1], F32, kind="ExternalInput")
    lb3_d = nc.dram_tensor("lb3", [1, 1], F32, kind="ExternalInput")
    out_d = nc.dram_tensor("out", [1, DPAD], F32, kind="ExternalOutput")

    with tile.TileContext(nc) as tc:
        with (
            tc.tile_pool(name="const", bufs=1) as cpool,
            tc.tile_pool(name="tblp", bufs=2) as tblp,
            tc.tile_pool(name="aggp", bufs=1) as aggp,
            tc.tile_pool(name="chk", bufs=2) as chk,
            tc.tile_pool(name="meta", bufs=2) as mpool,
            tc.tile_pool(name="ext", bufs=2) as epool,
            tc.tile_pool(name="epi", bufs=1) as epip,
            tc.tile_pool(name="work", bufs=2) as pool,
            tc.tile_pool(name="psA", bufs=2, space="PSUM") as psA,
            tc.tile_pool(name="psM", bufs=2, space="PSUM") as psM,
            tc.tile_pool(name="dram", bufs=1, space="DRAM") as dram,
        ):
            def load_const(dram_t, shape, dtype, tag):
                t = cpool.tile(shape, dtype, tag=tag)
                nc.sync.dma_start(t[:], dram_t[:])
                return t

            w1_t = load_const(w1_d, [128, 64], BF16, "w1")
            w2_t = load_const(w2_d, [64, 64], BF16, "w2")
            lw1_t = load_const(lw1_d, [64, 64], F32, "lw1")
            lw2_t = load_const(lw2_d, [64, 32], F32, "lw2")
            lw3_t = load_const(lw3_d, [32, 1], F32, "lw3")
            b1q_t = load_const(b1q_d, [128, 4], F32, "b1q")
            b2q_t = load_const(b2q_d, [128, 4], F32, "b2q")
            lb1_t = load_const(lb1_d, [64, 1], F32, "lb1")
            lb2_t = load_const(lb2_d, [32, 1], F32, "lb2")
            lb3_t = load_const(lb3_d, [1, 1], F32, "lb3")

            m1shA = dram.tile([16, WIN, 4], BF16)
            m1shB = dram.tile([16, WIN, 4], BF16)
            m2shA = dram.tile([16, WIN, 4], BF16)
            m2shB = dram.tile([16, WIN, 4], BF16)
            h1db = dram.tile([16, 4, DPAD], BF16)
            h2b = dram.tile([16, 4, DPAD], F32)
            m1Ta = dram.tile([NCORES * 16, WIN, 4], BF16, addr_space="Shared")
            m1Tb = dram.tile([NCORES * 16, WIN, 4], BF16, addr_space="Shared")
            m2Ta = dram.tile([NCORES * 16, WIN, 4], BF16, addr_space="Shared")
            m2Tb = dram.tile([NCORES * 16, WIN, 4], BF16, addr_space="Shared")

            agg = aggp.tile([128, SDPAD, 4], F32, tag="agg")

            def emit_msgs(lhsT_full, rhs_cols_cb, outA, outB, agA, agB):
                """Quad-split matmuls of rhs columns into half-shards; each
                half AllGathers as soon as it completes so the collective
                overlaps the other half's compute / the conv's first window."""
                for half, outT in ((0, outA), (1, outB)):
                    base = half * WIN
                    o = 0
                    while o < WIN:
                        cw = min(EC, WIN - o)
                        rhs = rhs_cols_cb(base + o, cw)
                        stg = pool.tile([16, EC, 4], BF16, tag="stg")
                        for j in range(4):
                            ps = psA.tile([16, EC], F32, tag="psa")
                            nc.tensor.matmul(
                                ps[:, :cw],
                                lhsT=lhsT_full[:, 16 * j:16 * j + 16],
                                rhs=rhs, start=True, stop=True)
                            nc.scalar.activation(
                                stg[:, :cw, j], ps[:, :cw],
                                mybir.ActivationFunctionType.Copy)
                        nc.sync.dma_start(outT[:, o:o + cw, :], stg[:, :cw, :])
                        o += cw
                    ag = agA if half == 0 else agB
                    nc.gpsimd.collective_compute(
                        "AllGather", mybir.AluOpType.bypass,
                        ins=[outT[:].opt()],
                        outs=[ag[:].opt()],
                        replica_groups=[list(range(NCORES))],
                    )

            # --- phase A: m1sh = W1^T xt (own block, quad layout) ---
            def xt_cols(o, cw):
                st = pool.tile([128, EC], BF16, tag="xt")
                nc.sync.dma_start(st[:, :cw], xt[:, o:o + cw])
                return st[:, :cw]

            emit_msgs(w1_t, xt_cols, m1shA, m1shB, m1Ta, m1Tb)

            def conv(msgTab, selfab, out_cb):
                """Software-pipelined: gather(i+1) is issued before
                extract(i) so the GpSimd queue never waits on the scans.
                Self-loop terms come from the local shard as contiguous
                slice-adds (never gathered)."""
                nc.vector.memset(agg[:], 0.0)
                for sh in range(4):
                    sd0 = sh * 400
                    slt = epip.tile([128, 400, 4], BF16, tag="slt")
                    for g in range(NSTREAM):
                        co = g * SDPAD + sd0
                        sf = selfab[co // WIN]
                        nc.sync.dma_start(
                            slt[16 * g:16 * g + 16, :, :],
                            sf[:, co % WIN:co % WIN + 400, :])
                    nc.vector.tensor_tensor(
                        agg[:, sd0:sd0 + 400, :], agg[:, sd0:sd0 + 400, :],
                        slt[:], op=mybir.AluOpType.add)
                calls = [(w, h) for w in range(NWIN) for h in range(NH)]
                goffs = np.concatenate(
                    [[0], np.cumsum(csz.reshape(-1))]).astype(int)
                tbls = {}
                stash = {}

                def emit_gather(i):
                    w, h = calls[i]
                    if h == 0:
                        cb, hb = w // 2, w % 2
                        tbl = tblp.tile([128, WIN, 4], BF16, tag="tbl")
                        for g in range(NSTREAM):
                            # scalar-issued queue + half-column splits: table
                            # bursts must not delay the gather-critical meta
                            # loads (shorter packets bound the collision wait)
                            for hc in range(2):
                                c0 = hc * (WIN // 2)
                                nc.scalar.dma_start(
                                    tbl[16 * g:16 * g + 16,
                                        c0:c0 + WIN // 2, :],
                                    msgTab[hb][16 * cb:16 * cb + 16,
                                               c0:c0 + WIN // 2, :])
                        tbls[w] = tbl
                    cs = int(csz[w, h])
                    goff = int(goffs[i])
                    it = mpool.tile([128, CSMAX // 16], I16, tag="gi")
                    nc.sync.dma_start(it[:, :cs // 16],
                                      gidx_d[:, goff // 16:(goff + cs) // 16])
                    mt = mpool.tile([128, CSMAX], BF16, tag="mk")
                    nc.sync.dma_start(mt[:, :cs], mask_d[:, goff:goff + cs])
                    gch = chk.tile([128, CSMAX, 4], BF16, tag="gch")
                    nc.gpsimd.ap_gather(
                        gch[:, :cs, :], tbls[w][:], it[:, :cs // 16],
                        channels=128, num_elems=WIN, d=4, num_idxs=cs)
                    stash[i] = (gch, mt, cs)

                emit_gather(0)
                for i in range(len(calls)):
                    if i + 1 < len(calls):
                        emit_gather(i + 1)
                    w, h = calls[i]
                    gch, mt, cs = stash.pop(i)
                    for j in range(4):
                        nc.vector.tensor_tensor_scan(
                            gch[:, :cs, j], mt[:, :cs], gch[:, :cs, j],
                            0.0, op0=mybir.AluOpType.mult,
                            op1=mybir.AluOpType.add)
                    et = mpool.tile([128, HALF // 16], I16, tag="ei")
                    d0 = h * HALF
                    eo = w * SDPAD + d0
                    nc.sync.dma_start(
                        et[:], eidx_d[:, eo // 16:(eo + HALF) // 16])
                    ex = epool.tile([128, HALF, 4], BF16, tag="ex")
                    nc.gpsimd.ap_gather(
                        ex[:], gch[:, :cs, :], et[:],
                        channels=128, num_elems=cs, d=4, num_idxs=HALF)
                    nc.vector.tensor_tensor(
                        agg[:, d0:d0 + HALF, :], agg[:, d0:d0 + HALF, :],
                        ex[:], op=mybir.AluOpType.add)
                out_cb()

            # --- conv1 epilogue: h1d = dinv*(dinv*agg + b1) -> h1db ---
            def conv1_out():
                for h in range(NH):
                    d0 = h * HALF
                    dq = epip.tile([128, HALF], F32, tag="dq")
                    nc.sync.dma_start(dq[:], dinvd_d[:, d0:d0 + HALF])
                    e1 = epip.tile([128, HALF, 4], F32, tag="e1")
                    nc.vector.tensor_tensor(
                        e1[:], agg[:, d0:d0 + HALF, :],
                        dq[:].unsqueeze(2).broadcast_to([128, HALF, 4]),
                        op=mybir.AluOpType.mult)
                    nc.vector.tensor_tensor(
                        e1[:], e1[:],
                        b1q_t[:].unsqueeze(1).broadcast_to([128, HALF, 4]),
                        op=mybir.AluOpType.add)
                    e2 = epip.tile([128, 4, HALF], BF16, tag="e2")
                    nc.vector.tensor_tensor(
                        e2[:], e1[:].transpose([0, 2, 1]),
                        dq[:].unsqueeze(1).broadcast_to([128, 4, HALF]),
                        op=mybir.AluOpType.mult)
                    for g in range(NSTREAM):
                        nc.sync.dma_start(
                            h1db[:, :, g * SDPAD + d0:g * SDPAD + d0 + HALF],
                            e2[16 * g:16 * g + 16, :, :].opt())

            conv((m1Ta, m1Tb), (m1shA, m1shB), conv1_out)

            # --- phase C: m2sh = W2^T h1d (own block) ---
            def h1_cols(o, cw):
                st = pool.tile([64, EC], BF16, tag="h1c")
                nc.sync.dma_start(st[:, :cw], h1db[:, :, o:o + cw])
                return st[:, :cw]

            emit_msgs(w2_t, h1_cols, m2shA, m2shB, m2Ta, m2Tb)

            # --- conv2 epilogue: h2 = dinv*agg + b2 -> h2b (f32) ---
            def conv2_out():
                for h in range(NH):
                    d0 = h * HALF
                    dq = epip.tile([128, HALF], F32, tag="dq")
                    nc.sync.dma_start(dq[:], dinvd_d[:, d0:d0 + HALF])
                    e1 = epip.tile([128, HALF, 4], F32, tag="e1")
                    nc.vector.tensor_tensor(
                        e1[:], agg[:, d0:d0 + HALF, :],
                        dq[:].unsqueeze(2).broadcast_to([128, HALF, 4]),
                        op=mybir.AluOpType.mult)
                    e2f = epip.tile([128, 4, HALF], F32, tag="e2f")
                    nc.vector.tensor_tensor(
                        e2f[:], e1[:].transpose([0, 2, 1]),
                        b2q_t[:].unsqueeze(2).broadcast_to([128, 4, HALF]),
                        op=mybir.AluOpType.add)
                    for g in range(NSTREAM):
                        nc.sync.dma_start(
                            h2b[:, :, g * SDPAD + d0:g * SDPAD + d0 + HALF],
                            e2f[16 * g:16 * g + 16, :, :].opt())

            conv((m2Ta, m2Tb), (m2shA, m2shB), conv2_out)

            # --- MLP head (transposed space, h2 streamed from DRAM) ---
            for o in range(0, DPAD, EC):
                w_ = min(EC, DPAD - o)
                h2c = pool.tile([64, EC], F32, tag="h2c")
                nc.sync.dma_start(h2c[:, :w_], h2b[:, :, o:o + w_])
                p1 = psM.tile([64, EC], F32, tag="mm1")
                nc.tensor.matmul(p1[:, :w_], lhsT=lw1_t[:],
                                 rhs=h2c[:, :w_], start=True, stop=True)
                z1 = pool.tile([64, EC], F32, tag="z1")
                nc.scalar.activation(z1[:, :w_], p1[:, :w_],
                                     mybir.ActivationFunctionType.Relu,
                                     bias=lb1_t[:])
                p2 = psM.tile([32, EC], F32, tag="mm2")
                nc.tensor.matmul(p2[:, :w_], lhsT=lw2_t[:], rhs=z1[:, :w_],
                                 start=True, stop=True)
                z2 = pool.tile([32, EC], F32, tag="z2")
                nc.scalar.activation(z2[:, :w_], p2[:, :w_],
                                     mybir.ActivationFunctionType.Relu,
                                     bias=lb2_t[:])
                p3 = psM.tile([1, EC], F32, tag="mm3")
                nc.tensor.matmul(p3[:, :w_], lhsT=lw3_t[:], rhs=z2[:, :w_],
                                 start=True, stop=True)
                z3 = pool.tile([1, EC], F32, tag="z3")
                nc.vector.tensor_tensor(z3[:, :w_], p3[:, :w_],
                                        lb3_t[:].broadcast_to([1, w_]),
                                        op=mybir.AluOpType.add)
                nc.sync.dma_start(out_d[:, o:o + w_], z3[:, :w_])

    nc.compile()
    return nc


def kernel(x, edge_index, W1, b1, W2, b2, lw1, lb1, lw2, lb2, lw3, lb3,
           _want_trace=False):
    x = np.asarray(x, np.float32)
    edge_index = np.asarray(edge_index)
    n = x.shape[0]

    dinv, plan, cores = preprocess(n, edge_index)

    # per-core xt: own nodes at padded positions, dinv-prescaled
    xs = (x * dinv[:, None]).astype(np.float32)

    # h1db/h2b row r = 4p+j holds feat p+16j -> permute consumer weight rows
    perm = np.array([(r // 4) + 16 * (r % 4) for r in range(64)])

    def quadb(b):
        q = np.asarray(b, np.float32).reshape(4, 16).T  # [p, j]
        return np.ascontiguousarray(np.tile(q, (8, 1)))

    posmap = plan["posmap"]
    in_maps = []
    for c in range(NCORES):
        xt = np.zeros((128, DPAD), ml_dtypes.bfloat16)
        lo, hi = c * SHARD, min((c + 1) * SHARD, n)
        xt[:, posmap[lo:hi]] = xs[lo:hi].T.astype(ml_dtypes.bfloat16)
        in_maps.append({
            "xt": xt,
            "gidx": cores[c]["gidx"], "mask": cores[c]["mask"],
            "eidx": cores[c]["eidx"],
            "dinvd": cores[c]["dinvd"],
            "w1": np.asarray(W1, np.float32).astype(ml_dtypes.bfloat16),
            "w2": np.ascontiguousarray(
                np.asarray(W2, np.float32)[perm]).astype(ml_dtypes.bfloat16),
            "lw1": np.ascontiguousarray(np.asarray(lw1, np.float32)[perm]),
            "lw2": np.ascontiguousarray(np.asarray(lw2, np.float32)),
            "lw3": np.ascontiguousarray(np.asarray(lw3, np.float32)),
            "b1q": quadb(b1), "b2q": quadb(b2),
            "lb1": np.asarray(lb1, np.float32).reshape(-1, 1),
            "lb2": np.asarray(lb2, np.float32).reshape(-1, 1),
            "lb3": np.asarray(lb3, np.float32).reshape(-1, 1),
        })

    nc = build_program(plan)
    res = run_bass_kernel_spmd(nc, in_maps, core_ids=list(range(NCORES)),
                               trace=_want_trace)
    out = np.empty((n, 1), np.float32)
    for c in range(NCORES):
        o = res.results[c]["out"][0]
        lo, hi = c * SHARD, min((c + 1) * SHARD, n)
        out[lo:hi, 0] = o[posmap[lo:hi]]
    kernel._last_exec_ns = res.exec_time_ns
    return out


# revision 32
# speedup vs baseline: 1.0202x; 1.0202x over previous
"""GCN (2x GCNConv + MLP head) on 8 TRN2 NeuronCores via Bass/Tile.

v4: sharded message build + split AllGathers + balanced scan aggregation.
  - Nodes owned by core in 12720-blocks. Each dst gets a SLOT via per-core
    degree-round-robin binning into 32 (stream, quarter) bins (balance_bins)
    so per-call gather counts are near the statistical floor. Table column
    of node v = 12800*c + posmap[v] EVERYWHERE (xt, m1T/m2T, h1db, agg,
    output). Slot 399 of every quarter is reserved empty -> every window
    has a zero column for gather pads (ZPAD).
  - Self-loops are never gathered: in slot layout their contribution is a
    contiguous slice of the core's LOCAL half-shards (m1shA/B, m2shA/B),
    added into agg with one DVE add per quarter at conv start.
  - Phase A computes W1^T xt for the own 12800-column block in two 6400
    halves; each half AllGathers immediately so the collective overlaps
    the other half's compute and the conv's first window. Same for phase C
    (W2^T h1d). PSUM->SBUF casts run on the Scalar engine.
  - conv: 16 windows x 4 dst-quarter calls, window tables double-buffered.
    Per call: one ap_gather (main), 4 per-quad tensor_tensor_scans
    (running masked segment sums), one extraction ap_gather (per-dst last
    positions; absent dsts point at a zeroed pad slot), one accumulate add.
    Calls software-pipelined: gather(i+1) issues before extract(i).
    ap_gather costs ~28ns/idx of serial GpSimd time (order/dtype/d
    independent) -- the dominant cost; all layout choices minimize total
    index count (edges + NWIN*SDPAD extraction slots).
  - conv1 epilogue -> h1db (dinv-prescaled bf16), conv2 epilogue -> h2b
    f32, MLP head on own shard, host reassembles via posmap.
"""

import numpy as np
import ml_dtypes

import concourse.bass as bass  # noqa: F401
import concourse.bacc as bacc
import concourse.tile as tile
import concourse.mybir as mybir
from concourse.bass_utils import run_bass_kernel_spmd

F32 = mybir.dt.float32
BF16 = mybir.dt.bfloat16
I16 = mybir.dt.int16

NCORES = 8
NSTREAM = 8
SHARD = 12720        # nodes owned per core (last core partial)
SDST = 1590          # nodes per dst-stream
SDPAD = 1600         # padded dsts per stream
NH = 4               # dst-quarter calls per window
HALF = SDPAD // NH   # 400
DPAD = SDPAD * NSTREAM          # 12800 table columns per core block
NPAD = DPAD * NCORES            # 102400 total table columns
WIN = 6400           # window width (2 windows per core block)
NWIN = NPAD // WIN   # 16
ZPAD = 399           # slot 399 of the window-first quarter is always zero
EC = 512             # column chunk for PE phases
N_FULL = 100000


def wrapg(a):
    """[8, n] per-group idx streams -> [128, n//16] int16 ap_gather layout."""
    out = np.empty((128, a.shape[1] // 16), np.int16)
    for g in range(NSTREAM):
        out[16 * g:16 * g + 16] = np.ascontiguousarray(a[g].reshape(-1, 16).T)
    return out


def col_of(v):
    """Table column of node id v (vectorized)."""
    c = v // SHARD
    l = v - c * SHARD
    g = l // SDST
    p = l - g * SDST
    return (c * DPAD + g * SDPAD + p), c, g, p


def balance_bins(n, indeg):
    """Per core, deal dsts round-robin (by in-degree, desc) into the 32
    (stream, quarter) bins; slot 399 of every quarter stays empty so each
    window has a guaranteed zero column for gather pads."""
    posmap = np.empty(n, np.int64)
    nbins = NSTREAM * NH
    for c in range(NCORES):
        lo, hi = c * SHARD, min((c + 1) * SHARD, n)
        m = hi - lo
        order_d = np.argsort(-indeg[lo:hi], kind="stable")
        b = np.arange(m) % nbins
        slot = np.arange(m) // nbins         # <= 397 < 399
        pos = (b // NH) * SDPAD + (b % NH) * HALF + slot
        pm = np.empty(m, np.int64)
        pm[order_d] = pos
        posmap[lo:hi] = pm
    return posmap


def preprocess(n, edge_index):
    """Slot-based layout: table column of node v = (v//SHARD)*DPAD +
    posmap[v] everywhere (xt, m1T, m2T, h1db, agg, output). Self-loops are
    NOT gathered -- their contribution is a contiguous slice-add from the
    core's local m1sh/m2sh. One shared metadata set serves both convs."""
    src = edge_index[0].astype(np.int64)
    dst = edge_index[1].astype(np.int64)

    deg = np.bincount(dst, minlength=n).astype(np.float64) + 1.0
    dinv = (1.0 / np.sqrt(deg)).astype(np.float32)

    indeg = np.bincount(dst, minlength=n)
    posmap = balance_bins(n, indeg)
    cpos = (np.arange(n) // SHARD) * DPAD + posmap

    srcA, dstA = src, dst                    # no self-loops in the gather
    colS = cpos[srcA]
    w_of = colS // WIN                       # 0..NWIN-1

    c_of = dstA // SHARD
    pD = posmap[dstA]                        # slot in [0, DPAD)
    g_of = pD // SDPAD
    sl = pD % SDPAD                          # position within stream
    h_of = sl // HALF                        # 0..3

    key = (((c_of * NSTREAM + g_of) * NWIN + w_of) * NH + h_of)
    order = np.lexsort((sl, key))
    srcC, slS = colS[order], sl[order]

    ncalls = NWIN * NH
    counts = np.bincount(key, minlength=NCORES * NSTREAM * ncalls)
    starts = np.zeros(len(counts) + 1, np.int64)
    np.cumsum(counts, out=starts[1:])
    cmax = counts.reshape(NCORES, NSTREAM, NWIN, NH).max(axis=(0, 1))
    csz = ((cmax + 1 + 15) // 16) * 16       # >= cmax+1: room for a pad slot
    gtot = int(csz.sum())
    etot = NWIN * SDPAD

    cores = []
    for c in range(NCORES):
        gidx = np.full((NSTREAM, gtot), ZPAD, np.int16)   # pads -> zero col
        mask = np.ones((NSTREAM, gtot), np.float32)
        eidx = np.zeros((NSTREAM, etot), np.int16)
        off = 0
        eoff = 0
        for w in range(NWIN):
            for h in range(NH):
                cs = int(csz[w, h])
                d0 = h * HALF
                for g in range(NSTREAM):
                    k = ((c * NSTREAM + g) * NWIN + w) * NH + h
                    a, b = starts[k], starts[k + 1]
                    cnt = b - a
                    d_loc = slS[a:b]
                    gidx[g, off:off + cnt] = (srcC[a:b] - w * WIN).astype(np.int16)
                    m = np.ones(cnt + 1, np.float32)
                    m[0] = 0.0
                    if cnt:
                        m[1:cnt][d_loc[1:] != d_loc[:-1]] = 0.0
                        m[cnt] = 0.0          # first pad slot resets to 0
                        lastpos = np.full(HALF, cnt, np.int64)  # absent -> pad
                        lastpos[d_loc - d0] = np.arange(cnt)    # last wins
                        eidx[g, eoff + d0:eoff + d0 + HALF] = lastpos.astype(np.int16)
                    else:
                        eidx[g, eoff + d0:eoff + d0 + HALF] = 0
                    mask[g, off:off + cnt + 1] = m
                off += cs
            eoff += SDPAD
        dinvd = np.zeros(DPAD, np.float32)
        lo, hi = c * SHARD, min((c + 1) * SHARD, n)
        dinvd[posmap[lo:hi]] = dinv[lo:hi]
        dinvd = dinvd.reshape(NSTREAM, SDPAD)
        cores.append(dict(
            gidx=wrapg(gidx),
            mask=np.repeat(mask, 16, axis=0).astype(ml_dtypes.bfloat16),
            eidx=wrapg(eidx),
            dinvd=np.repeat(dinvd, 16, axis=0),
        ))

    plan = dict(csz=csz, gtot=gtot, etot=etot, posmap=posmap)
    return dinv, plan, cores


def build_program(plan):
    csz = plan["csz"]
    gtot = plan["gtot"]
    etot = plan["etot"]
    CSMAX = int(csz.max())

    nc = bacc.Bacc("TRN2", target_bir_lowering=False, debug=False,
                   num_devices=NCORES)

    xt = nc.dram_tensor("xt", [128, DPAD], BF16, kind="ExternalInput")
    gidx_d = nc.dram_tensor("gidx", [128, gtot // 16], I16, kind="ExternalInput")
    mask_d = nc.dram_tensor("mask", [128, gtot], BF16, kind="ExternalInput")
    eidx_d = nc.dram_tensor("eidx", [128, etot // 16], I16, kind="ExternalInput")
    dinvd_d = nc.dram_tensor("dinvd", [128, SDPAD], F32, kind="ExternalInput")
    w1_d = nc.dram_tensor("w1", [128, 64], BF16, kind="ExternalInput")
    w2_d = nc.dram_tensor("w2", [64, 64], BF16, kind="ExternalInput")
    lw1_d = nc.dram_tensor("lw1", [64, 64], F32, kind="ExternalInput")
    lw2_d = nc.dram_tensor("lw2", [64, 32], F32, kind="ExternalInput")
    lw3_d = nc.dram_tensor("lw3", [32, 1], F32, kind="ExternalInput")
    b1q_d = nc.dram_tensor("b1q", [128, 4], F32, kind="ExternalInput")
    b2q_d = nc.dram_tensor("b2q", [128, 4], F32, kind="ExternalInput")
    lb1_d = nc.dram_tensor("lb1", [64, 1], F32, kind="ExternalInput")
    lb2_d = nc.dram_tensor("lb2", [32, 1], F32, kind="ExternalInput")
    lb3_d = nc.dram_tensor("lb3", [1, 1], F32, kind="ExternalInput")
    out_d = nc.dram_tensor("out", [1, DPAD], F32, kind="ExternalOutput")

    with tile.TileContext(nc) as tc:
        with (
            tc.tile_pool(name="const", bufs=1) as cpool,
            tc.tile_pool(name="tblp", bufs=2) as tblp,
            tc.tile_pool(name="aggp", bufs=1) as aggp,
            tc.tile_pool(name="chk", bufs=2) as chk,
            tc.tile_pool(name="meta", bufs=2) as mpool,
            tc.tile_pool(name="ext", bufs=2) as epool,
            tc.tile_pool(name="epi", bufs=1) as epip,
            tc.tile_pool(name="work", bufs=2) as pool,
            tc.tile_pool(name="psA", bufs=2, space="PSUM") as psA,
            tc.tile_pool(name="psM", bufs=2, space="PSUM") as psM,
            tc.tile_pool(name="dram", bufs=1, space="DRAM") as dram,
        ):
            def load_const(dram_t, shape, dtype, tag):
                t = cpool.tile(shape, dtype, tag=tag)
                nc.sync.dma_start(t[:], dram_t[:])
                return t

            w1_t = load_const(w1_d, [128, 64], BF16, "w1")
            w2_t = load_const(w2_d, [64, 64], BF16, "w2")
            lw1_t = load_const(lw1_d, [64, 64], F32, "lw1")
            lw2_t = load_const(lw2_d, [64, 32], F32, "lw2")
            lw3_t = load_const(lw3_d, [32, 1], F32, "lw3")
            b1q_t = load_const(b1q_d, [128, 4], F32, "b1q")
            b2q_t = load_const(b2q_d, [128, 4], F32, "b2q")
            lb1_t = load_const(lb1_d, [64, 1], F32, "lb1")
            lb2_t = load_const(lb2_d, [32, 1], F32, "lb2")
            lb3_t = load_const(lb3_d, [1, 1], F32, "lb3")

            m1shA = dram.tile([16, WIN, 4], BF16)
            m1shB = dram.tile([16, WIN, 4], BF16)
            m2shA = dram.tile([16, WIN, 4], BF16)
            m2shB = dram.tile([16, WIN, 4], BF16)
            h1db = dram.tile([16, 4, DPAD], BF16)
            h2b = dram.tile([16, 4, DPAD], F32)
            m1Ta = dram.tile([NCORES * 16, WIN, 4], BF16, addr_space="Shared")
            m1Tb = dram.tile([NCORES * 16, WIN, 4], BF16, addr_space="Shared")
            m2Ta = dram.tile([NCORES * 16, WIN, 4], BF16, addr_space="Shared")
            m2Tb = dram.tile([NCORES * 16, WIN, 4], BF16, addr_space="Shared")

            agg = aggp.tile([128, SDPAD, 4], F32, tag="agg")

            def emit_msgs(lhsT_full, rhs_cols_cb, outA, outB, agA, agB):
                """Quad-split matmuls of rhs columns into half-shards; each
                half AllGathers as soon as it completes so the collective
                overlaps the other half's compute / the conv's first window."""
                for half, outT in ((0, outA), (1, outB)):
                    base = half * WIN
                    o = 0
                    while o < WIN:
                        cw = min(EC, WIN - o)
                        rhs = rhs_cols_cb(base + o, cw)
                        stg = pool.tile([16, EC, 4], BF16, tag="stg")
                        for j in range(4):
                            ps = psA.tile([16, EC], F32, tag="psa")
                            nc.tensor.matmul(
                                ps[:, :cw],
                                lhsT=lhsT_full[:, 16 * j:16 * j + 16],
                                rhs=rhs, start=True, stop=True)
                            nc.scalar.activation(
                                stg[:, :cw, j], ps[:, :cw],
                                mybir.ActivationFunctionType.Copy)
                        nc.sync.dma_start(outT[:, o:o + cw, :], stg[:, :cw, :])
                        o += cw
                    ag = agA if half == 0 else agB
                    nc.gpsimd.collective_compute(
                        "AllGather", mybir.AluOpType.bypass,
                        ins=[outT[:].opt()],
                        outs=[ag[:].opt()],
                        replica_groups=[list(range(NCORES))],
                    )

            # --- phase A: m1sh = W1^T xt (own block, quad layout) ---
            def xt_cols(o, cw):
                st = pool.tile([128, EC], BF16, tag="xt")
                nc.sync.dma_start(st[:, :cw], xt[:, o:o + cw])
                return st[:, :cw]

            emit_msgs(w1_t, xt_cols, m1shA, m1shB, m1Ta, m1Tb)

            def conv(msgTab, selfab, out_cb):
                """Software-pipelined: gather(i+1) is issued before
                extract(i) so the GpSimd queue never waits on the scans.
                Self-loop terms come from the local shard as contiguous
                slice-adds (never gathered)."""
                nc.vector.memset(agg[:], 0.0)
                for sh in range(4):
                    sd0 = sh * 400
                    slt = epip.tile([128, 400, 4], BF16, tag="slt")
                    for g in range(NSTREAM):
                        co = g * SDPAD + sd0
                        sf = selfab[co // WIN]
                        nc.sync.dma_start(
                            slt[16 * g:16 * g + 16, :, :],
                            sf[:, co % WIN:co % WIN + 400, :])
                    nc.vector.tensor_tensor(
                        agg[:, sd0:sd0 + 400, :], agg[:, sd0:sd0 + 400, :],
                        slt[:], op=mybir.AluOpType.add)
                calls = [(w, h) for w in range(NWIN) for h in range(NH)]
                goffs = np.concatenate(
                    [[0], np.cumsum(csz.reshape(-1))]).astype(int)
                tbls = {}
                stash = {}

                def emit_gather(i):
                    w, h = calls[i]
                    if h == 0:
                        cb, hb = w // 2, w % 2
                        tbl = tblp.tile([128, WIN, 4], BF16, tag="tbl")
                        for g in range(NSTREAM):
                            # scalar-issued queue: table bursts must not
                            # delay the latency-critical meta loads on sync
                            nc.scalar.dma_start(
                                tbl[16 * g:16 * g + 16, :, :],
                                msgTab[hb][16 * cb:16 * cb + 16, :, :])
                        tbls[w] = tbl
                    cs = int(csz[w, h])
                    goff = int(goffs[i])
                    it = mpool.tile([128, CSMAX // 16], I16, tag="gi")
                    nc.sync.dma_start(it[:, :cs // 16],
                                      gidx_d[:, goff // 16:(goff + cs) // 16])
                    mt = mpool.tile([128, CSMAX], BF16, tag="mk")
                    nc.sync.dma_start(mt[:, :cs], mask_d[:, goff:goff + cs])
                    gch = chk.tile([128, CSMAX, 4], BF16, tag="gch")
                    nc.gpsimd.ap_gather(
                        gch[:, :cs, :], tbls[w][:], it[:, :cs // 16],
                        channels=128, num_elems=WIN, d=4, num_idxs=cs)
                    et = mpool.tile([128, HALF // 16], I16, tag="ei")
                    eo = w * SDPAD + h * HALF
                    nc.sync.dma_start(
                        et[:], eidx_d[:, eo // 16:(eo + HALF) // 16])
                    stash[i] = (gch, mt, cs, et)

                emit_gather(0)
                for i in range(len(calls)):
                    if i + 1 < len(calls):
                        emit_gather(i + 1)
                    w, h = calls[i]
                    gch, mt, cs, et = stash.pop(i)
                    for j in range(4):
                        nc.vector.tensor_tensor_scan(
                            gch[:, :cs, j], mt[:, :cs], gch[:, :cs, j],
                            0.0, op0=mybir.AluOpType.mult,
                            op1=mybir.AluOpType.add)
                    d0 = h * HALF
                    ex = epool.tile([128, HALF, 4], BF16, tag="ex")
                    nc.gpsimd.ap_gather(
                        ex[:], gch[:, :cs, :], et[:],
                        channels=128, num_elems=cs, d=4, num_idxs=HALF)
                    nc.vector.tensor_tensor(
                        agg[:, d0:d0 + HALF, :], agg[:, d0:d0 + HALF, :],
                        ex[:], op=mybir.AluOpType.add)
                out_cb()

            # --- conv1 epilogue: h1d = dinv*(dinv*agg + b1) -> h1db ---
            def conv1_out():
                for h in range(NH):
                    d0 = h * HALF
                    dq = epip.tile([128, HALF], F32, tag="dq")
                    nc.sync.dma_start(dq[:], dinvd_d[:, d0:d0 + HALF])
                    e1 = epip.tile([128, HALF, 4], F32, tag="e1")
                    nc.vector.tensor_tensor(
                        e1[:], agg[:, d0:d0 + HALF, :],
                        dq[:].unsqueeze(2).broadcast_to([128, HALF, 4]),
                        op=mybir.AluOpType.mult)
                    nc.vector.tensor_tensor(
                        e1[:], e1[:],
                        b1q_t[:].unsqueeze(1).broadcast_to([128, HALF, 4]),
                        op=mybir.AluOpType.add)
                    e2 = epip.tile([128, 4, HALF], BF16, tag="e2")
                    nc.vector.tensor_tensor(
                        e2[:], e1[:].transpose([0, 2, 1]),
                        dq[:].unsqueeze(1).broadcast_to([128, 4, HALF]),
                        op=mybir.AluOpType.mult)
                    for g in range(NSTREAM):
                        nc.sync.dma_start(
                            h1db[:, :, g * SDPAD + d0:g * SDPAD + d0 + HALF],
                            e2[16 * g:16 * g + 16, :, :].opt())

            conv((m1Ta, m1Tb), (m1shA, m1shB), conv1_out)

            # --- phase C: m2sh = W2^T h1d (own block) ---
            def h1_cols(o, cw):
                st = pool.tile([64, EC], BF16, tag="h1c")
                nc.sync.dma_start(st[:, :cw], h1db[:, :, o:o + cw])
                return st[:, :cw]

            emit_msgs(w2_t, h1_cols, m2shA, m2shB, m2Ta, m2Tb)

            # --- conv2 epilogue: h2 = dinv*agg + b2 -> h2b (f32) ---
            def conv2_out():
                for h in range(NH):
                    d0 = h * HALF
                    dq = epip.tile([128, HALF], F32, tag="dq")
                    nc.sync.dma_start(dq[:], dinvd_d[:, d0:d0 + HALF])
                    e1 = epip.tile([128, HALF, 4], F32, tag="e1")
                    nc.vector.tensor_tensor(
                        e1[:], agg[:, d0:d0 + HALF, :],
                        dq[:].unsqueeze(2).broadcast_to([128, HALF, 4]),
                        op=mybir.AluOpType.mult)
                    e2f = epip.tile([128, 4, HALF], F32, tag="e2f")
                    nc.vector.tensor_tensor(
                        e2f[:], e1[:].transpose([0, 2, 1]),
                        b2q_t[:].unsqueeze(2).broadcast_to([128, 4, HALF]),
                        op=mybir.AluOpType.add)
                    for g in range(NSTREAM):
                        nc.sync.dma_start(
                            h2b[:, :, g * SDPAD + d0:g * SDPAD + d0 + HALF],
                            e2f[16 * g:16 * g + 16, :, :].opt())

            conv((m2Ta, m2Tb), (m2shA, m2shB), conv2_out)

            # --- MLP head (transposed space, h2 streamed from DRAM) ---
            for o in range(0, DPAD, EC):
                w_ = min(EC, DPAD - o)
                h2c = pool.tile([64, EC], F32, tag="h2c")
                nc.sync.dma_start(h2c[:, :w_], h2b[:, :, o:o + w_])
                p1 = psM.tile([64, EC], F32, tag="mm1")
                nc.tensor.matmul(p1[:, :w_], lhsT=lw1_t[:],
                                 rhs=h2c[:, :w_], start=True, stop=True)
                z1 = pool.tile([64, EC], F32, tag="z1")
                nc.scalar.activation(z1[:, :w_], p1[:, :w_],
                                     mybir.ActivationFunctionType.Relu,
                                     bias=lb1_t[:])
                p2 = psM.tile([32, EC], F32, tag="mm2")
                nc.tensor.matmul(p2[:, :w_], lhsT=lw2_t[:], rhs=z1[:, :w_],
                                 start=True, stop=True)
                z2 = pool.tile([32, EC], F32, tag="z2")
                nc.scalar.activation(z2[:, :w_], p2[:, :w_],
                                     mybir.ActivationFunctionType.Relu,
                                     bias=lb2_t[:])
                p3 = psM.tile([1, EC], F32, tag="mm3")
                nc.tensor.matmul(p3[:, :w_], lhsT=lw3_t[:], rhs=z2[:, :w_],
                                 start=True, stop=True)
                z3 = pool.tile([1, EC], F32, tag="z3")
                nc.vector.tensor_tensor(z3[:, :w_], p3[:, :w_],
                                        lb3_t[:].broadcast_to([1, w_]),
                                        op=mybir.AluOpType.add)
                nc.sync.dma_start(out_d[:, o:o + w_], z3[:, :w_])

    nc.compile()
    return nc


def kernel(x, edge_index, W1, b1, W2, b2, lw1, lb1, lw2, lb2, lw3, lb3,
           _want_trace=False):
    x = np.asarray(x, np.float32)
    edge_index = np.asarray(edge_index)
    n = x.shape[0]

    dinv, plan, cores = preprocess(n, edge_index)

    # per-core xt: own nodes at padded positions, dinv-prescaled
    xs = (x * dinv[:, None]).astype(np.float32)

    # h1db/h2b row r = 4p+j holds feat p+16j -> permute consumer weight rows
    perm = np.array([(r // 4) + 16 * (r % 4) for r in range(64)])

    def quadb(b):
        q = np.asarray(b, np.float32).reshape(4, 16).T  # [p, j]
        return np.ascontiguousarray(np.tile(q, (8, 1)))

    posmap = plan["posmap"]
    in_maps = []
    for c in range(NCORES):
        xt = np.zeros((128, DPAD), ml_dtypes.bfloat16)
        lo, hi = c * SHARD, min((c + 1) * SHARD, n)
        xt[:, posmap[lo:hi]] = xs[lo:hi].T.astype(ml_dtypes.bfloat16)
        in_maps.append({
            "xt": xt,
            "gidx": cores[c]["gidx"], "mask": cores[c]["mask"],
            "eidx": cores[c]["eidx"],
            "dinvd": cores[c]["dinvd"],
            "w1": np.asarray(W1, np.float32).astype(ml_dtypes.bfloat16),
            "w2": np.ascontiguousarray(
                np.asarray(W2, np.float32)[perm]).astype(ml_dtypes.bfloat16),
            "lw1": np.ascontiguousarray(np.asarray(lw1, np.float32)[perm]),
            "lw2": np.ascontiguousarray(np.asarray(lw2, np.float32)),
            "lw3": np.ascontiguousarray(np.asarray(lw3, np.float32)),
            "b1q": quadb(b1), "b2q": quadb(b2),
            "lb1": np.asarray(lb1, np.float32).reshape(-1, 1),
            "lb2": np.asarray(lb2, np.float32).reshape(-1, 1),
            "lb3": np.asarray(lb3, np.float32).reshape(-1, 1),
        })

    nc = build_program(plan)
    res = run_bass_kernel_spmd(nc, in_maps, core_ids=list(range(NCORES)),
                               trace=_want_trace)
    out = np.empty((n, 1), np.float32)
    for c in range(NCORES):
        o = res.results[c]["out"][0]
        lo, hi = c * SHARD, min((c + 1) * SHARD, n)
        out[lo:hi, 0] = o[posmap[lo:hi]]
    kernel._last_exec_ns = res.exec_time_ns
    return out
